# revision 5
# baseline (speedup 1.0000x reference)
"""Trainium2 Bass kernel for nn_CrossModalAttention (sparse per-channel 3x3
token-window attention).

Contract: kernel(**inputs) takes the FULL fp32 inputs (B=8,C=256,H=W=64) and
returns the FULL fp32 output.  Internally: data-parallel over batch across the
8 NeuronCores (1 batch element per core), params replicated.

Per-core pipeline (all on one NeuronCore, emitted via Tile):
  PE    : Q/K/V 1x1-conv projections as bf16 matmuls (contraction over input
          channels, 2 k-halves of 128), PSUM accumulate.
  ACT   : PSUM->SBUF evacuation of K/V with fused bias-add + bf16 cast into a
          zero-padded 18x18 token grid; exp(); attention-weight broadcast over
          the 16 token pixels.
  DVE   : Q evacuation, Q*K neighbor products (bf16 2x), d-reduce via halving
          tree, softmax sum/reciprocal, attn*V products + accumulation,
          1/denominator scaling and fp32 residual add.
  DMA   : contiguous loads (host pre-tokenizes/pre-transposes), store.

Host side reorders pixels into token order ([C, nh, nw, ts, ts]) so every DMA
is contiguous and every neighbor shift is a pure free-dim offset on chip.
"""

import os
import sys
from contextlib import ExitStack

import numpy as np

for _p in ("/opt/trn_rl_repo",):
    if _p not in sys.path and os.path.isdir(_p):
        sys.path.insert(0, _p)

import ml_dtypes  # noqa: E402

import concourse.bacc as bacc  # noqa: E402
import concourse.bass as bass  # noqa: E402
import concourse.tile as tile  # noqa: E402
from concourse import mybir  # noqa: E402
from concourse.bass_utils import run_bass_kernel_spmd  # noqa: E402

BF16 = mybir.dt.bfloat16
F32 = mybir.dt.float32
ALU = mybir.AluOpType
ACTF = mybir.ActivationFunctionType

B, C, H, W = 8, 256, 64, 64
TS = 4                      # token size
NH = H // TS                # 16 token rows
NW = W // TS                # 16 token cols
T = NH * NW                 # 256 tokens
D = TS * TS                 # 16 pixels per token
G = 2                       # channel groups of 128
P = 128
PIX = H * W                 # 4096
GRID = NH + 2               # 18 (zero-padded token grid)
SCALE = float(D) ** -0.5    # 0.25
N_CORES = 8

_BUILT = None


def _emit(ctx: ExitStack, tc: "tile.TileContext"):
    nc = tc.nc

    # ---- DRAM I/O (per-core shard) ----
    xb_d = nc.dram_tensor("xb", [P, G, PIX], BF16, kind="ExternalInput").ap()
    xw_d = nc.dram_tensor("xw", [P, G, PIX], BF16, kind="ExternalInput").ap()
    w_d = nc.dram_tensor("wall", [P, 3, G, C], BF16, kind="ExternalInput").ap()
    b_d = nc.dram_tensor("ball", [P, 6], F32, kind="ExternalInput").ap()
    out_d = nc.dram_tensor("out", [P, G, PIX], F32, kind="ExternalOutput").ap()

    consts = ctx.enter_context(tc.tile_pool(name="consts", bufs=1))
    psum = ctx.enter_context(tc.tile_pool(name="psum", bufs=4, space="PSUM"))
    prod = ctx.enter_context(tc.tile_pool(name="prod", bufs=2))
    tree = ctx.enter_context(tc.tile_pool(name="tree", bufs=3))
    pexpp = ctx.enter_context(tc.tile_pool(name="pexp", bufs=2))
    accp = ctx.enter_context(tc.tile_pool(name="acc", bufs=2))
    outp = ctx.enter_context(tc.tile_pool(name="outf", bufs=1))

    # ---- persistent SBUF tiles ----
    xb = consts.tile([P, G, PIX], BF16)        # blue, token order
    xw = consts.tile([P, G, PIX], BF16)        # white, token order
    wall = consts.tile([P, 3, G, C], BF16)     # W^T tiles: [a_lo, proj, a_hi, c_out]
    ball = consts.tile([P, 6], F32)            # biases: col = proj*2 + grp
    qsb = consts.tile([P, G, PIX], BF16)
    kvp = consts.tile([P, 2, G, GRID, GRID, D], BF16)  # padded K (0) / V (1)
    LS = 10                                    # logit slot stride (9 used)
    lsb = consts.tile([P, G, T, LS], BF16)     # logits, 9 of 10 slots used
    sr = consts.tile([P, 2, G, T], F32)        # softmax sum / reciprocal

    nc.sync.dma_start(xb[:], xb_d[:])
    nc.sync.dma_start(xw[:], xw_d[:])
    nc.sync.dma_start(wall[:], w_d[:])
    nc.sync.dma_start(ball[:], b_d[:])

    # zero the 1-token border ring of the padded K/V grids
    for kv in range(2):
        for g in range(G):
            nc.gpsimd.memset(kvp[:, kv, g, 0:GRID:GRID - 1, :, :], 0.0)
            nc.gpsimd.memset(kvp[:, kv, g, :, 0:GRID:GRID - 1, :], 0.0)

    # ---- projections: out[c,pix] = sum_a W[c,a] x[a,pix] + b[c] ----
    # lhsT[k=a(128-half), m=c_out(128-group)], rhs[k, 512-col chunk] -> PSUM
    CH = 1024  # psum chunk (2 banks), 4 chunks per (proj, grp)
    for proj, src in ((0, xb), (1, xw), (2, xw)):
        for g in range(G):
            bias_ap = ball[:, proj * 2 + g: proj * 2 + g + 1]
            for u in range(PIX // CH):  # 4 chunks of 64 tokens (4 I-rows)
                pt = psum.tile([P, CH], F32)
                for j in range(CH // 512):
                    cols = slice(u * CH + j * 512, u * CH + (j + 1) * 512)
                    for h in range(2):
                        nc.tensor.matmul(
                            pt[:, j * 512:(j + 1) * 512],
                            wall[:, proj, h, g * P:(g + 1) * P],
                            src[:, h, cols],
                            start=(h == 0),
                            stop=(h == 1),
                        )
                if proj == 0:  # Q -> flat, DVE
                    nc.vector.tensor_scalar(
                        qsb[:, g, u * CH:(u + 1) * CH], pt[:], bias_ap, None,
                        op0=ALU.add,
                    )
                else:  # K/V -> padded grid interior, ACT (fused bias + cast)
                    dst = kvp[:, proj - 1, g, 1 + 4 * u:1 + 4 * u + 4, 1:1 + NW, :]
                    nc.scalar.activation(dst, pt[:], ACTF.Identity, bias=bias_ap)

    # ---- per-channel neighbor attention ----
    qv = [qsb[:, g, :].rearrange("p (t d) -> p t d", d=D) for g in range(G)]
    for g in range(G):
        # logits: l[c,t,n] = scale * sum_d q[c,t,d] k[c,t+dn,d]
        for di in range(3):
            for dj in range(3):
                n = di * 3 + dj
                kview = kvp[:, 0, g, di:di + NH, dj:dj + NW, :]
                pt = prod.tile([P, T, D], BF16, tag="prod")
                nc.vector.tensor_tensor(pt[:], qv[g], kview, op=ALU.mult)
                p1 = tree.tile([P, T, 8], BF16, tag="tree")
                nc.vector.tensor_tensor(p1[:], pt[:, :, 0:8], pt[:, :, 8:16], op=ALU.add)
                p2 = tree.tile([P, T, 4], BF16, tag="tree")
                nc.vector.tensor_tensor(p2[:], p1[:, :, 0:4], p1[:, :, 4:8], op=ALU.add)
                p3 = tree.tile([P, T, 2], BF16, tag="tree")
                nc.vector.tensor_tensor(p3[:], p2[:, :, 0:2], p2[:, :, 2:4], op=ALU.add)
                nc.vector.tensor_tensor(
                    lsb[:, g, :, n:n + 1], p3[:, :, 0:1], p3[:, :, 1:2], op=ALU.add
                )
        # softmax over the 9 neighbors (no max-subtraction: |logit| <~ 5)
        ev = lsb[:, g, :, 0:9]
        nc.scalar.activation(ev, ev, ACTF.Exp, scale=SCALE)
        nc.vector.tensor_reduce(sr[:, 0, g, :], ev, axis=mybir.AxisListType.X, op=ALU.add)
        nc.vector.reciprocal(sr[:, 1, g, :], sr[:, 0, g, :])

        # out[c,t,d] = (sum_n e_n[c,t] * v[c,t+dn,d]) * r[c,t] + blue[c,t,d]
        acc = accp.tile([P, T, D], BF16, tag="acc")
        for di in range(3):
            for dj in range(3):
                n = di * 3 + dj
                vview = kvp[:, 1, g, di:di + NH, dj:dj + NW, :]
                pe = pexpp.tile([P, T, D], BF16, tag="pexp")
                nc.scalar.activation(
                    pe[:], lsb[:, g, :, n:n + 1].broadcast_to([P, T, D]), ACTF.Copy
                )
                if n == 0:
                    nc.vector.tensor_tensor(acc[:], vview, pe[:], op=ALU.mult)
                else:
                    tn = prod.tile([P, T, D], BF16, tag="prod")
                    nc.vector.tensor_tensor(tn[:], vview, pe[:], op=ALU.mult)
                    nc.vector.tensor_tensor(acc[:], acc[:], tn[:], op=ALU.add)
        rview = sr[:, 1, g, :].unsqueeze(2).broadcast_to([P, T, D])
        nc.vector.tensor_tensor(acc[:], acc[:], rview, op=ALU.mult)
        of = outp.tile([P, PIX], F32, tag="outf")
        nc.vector.tensor_tensor(
            of[:], acc[:].rearrange("p t d -> p (t d)"), xb[:, g, :], op=ALU.add
        )
        nc.sync.dma_start(out_d[:, g, :], of[:])


def _build():
    global _BUILT
    if _BUILT is None:
        nc = bacc.Bacc(
            "TRN2", target_bir_lowering=False, debug=False, num_devices=N_CORES
        )
        with tile.TileContext(nc) as tc:
            with ExitStack() as ctx:
                _emit(ctx, tc)
        nc.compile()
        _BUILT = nc
    return _BUILT


def _tokenize(x: np.ndarray) -> np.ndarray:
    """[C,H,W] -> [C, nh*nw*ts*ts] in token order (I, J, u, v)."""
    c = x.shape[0]
    return (
        x.reshape(c, NH, TS, NW, TS).transpose(0, 1, 3, 2, 4).reshape(c, PIX)
    )


def _untokenize(y: np.ndarray) -> np.ndarray:
    """[C, PIX] token order -> [C, H, W]."""
    c = y.shape[0]
    return (
        y.reshape(c, NH, NW, TS, TS).transpose(0, 1, 3, 2, 4).reshape(c, H, W)
    )


def _part_fold(x: np.ndarray) -> np.ndarray:
    """[C, F] -> [P, C//P, F] partition-major fold."""
    return np.ascontiguousarray(
        x.reshape(C // P, P, -1).transpose(1, 0, 2)
    )


def _prep_maps(blue_feat, white_feat, Wq, bq, Wk, bk, Wv, bv):
    bf16 = ml_dtypes.bfloat16
    wall = np.stack([np.asarray(w, np.float32).T for w in (Wq, Wk, Wv)])  # [3,a,c]
    wall = np.ascontiguousarray(
        wall.reshape(3, 2, P, C).transpose(2, 0, 1, 3)
    ).astype(bf16)  # [P, 3, a_hi, c]
    ball = np.ascontiguousarray(
        np.stack([bq, bk, bv]).astype(np.float32).reshape(3, G, P).transpose(2, 0, 1)
    ).reshape(P, 6)
    maps = []
    for b in range(B):
        xb = _part_fold(_tokenize(np.asarray(blue_feat[b], np.float32))).astype(bf16)
        xw = _part_fold(_tokenize(np.asarray(white_feat[b], np.float32))).astype(bf16)
        maps.append({"xb": xb, "xw": xw, "wall": wall, "ball": ball})
    return maps


def _gather(results) -> np.ndarray:
    out = np.empty((B, C, H, W), np.float32)
    for b in range(B):
        y = results[b]["out"]  # [P, G, PIX] f32
        y = np.asarray(y, np.float32).transpose(1, 0, 2).reshape(C, PIX)
        out[b] = _untokenize(y)
    return out


def _install_ntff_hook():
    """The agent image's antenv lacks axon_hooks; synthesize it so
    run_bass_kernel_spmd(trace=True) can drive NTFF profiling via the
    injected libaxon_pjrt.so C ABI (mirrors trn_agent_boot.trn_boot)."""
    import contextlib
    import ctypes
    import types

    if "antenv.axon_hooks" in sys.modules:
        return
    so_path = "/opt/axon/libaxon_pjrt.so"
    lib = ctypes.CDLL(so_path)
    if not hasattr(lib, "axon_start_nrt_profile"):
        return
    lib.axon_start_nrt_profile.argtypes = [
        ctypes.POINTER(ctypes.c_int64),
        ctypes.c_size_t,
    ]
    lib.axon_start_nrt_profile.restype = ctypes.c_int64
    lib.axon_stop_nrt_profile.argtypes = [ctypes.c_char_p]
    lib.axon_stop_nrt_profile.restype = ctypes.c_int64

    @contextlib.contextmanager
    def _hook(output_dir, device_ids):
        import jax

        jax.devices()
        if device_ids:
            ids = (ctypes.c_int64 * len(device_ids))(*device_ids)
            rc = lib.axon_start_nrt_profile(ids, len(device_ids))
        else:
            rc = lib.axon_start_nrt_profile(None, 0)
        if rc != 0:
            raise RuntimeError(f"axon_start_nrt_profile rc={rc}")
        try:
            yield
        finally:
            n = lib.axon_stop_nrt_profile(str(output_dir).encode())
            print(f"ntff profile: {n} file(s) written to {output_dir}")

    mod = types.ModuleType("antenv.axon_hooks")
    mod.get_axon_ntff_profile_hook = lambda: _hook  # type: ignore[attr-defined]
    mod.set_axon_ntff_profile_hook = lambda h: None  # type: ignore[attr-defined]
    sys.modules["antenv.axon_hooks"] = mod


def run(trace=False, **inputs):
    nc = _build()
    maps = _prep_maps(**inputs)
    if trace:
        _install_ntff_hook()
    res = run_bass_kernel_spmd(nc, maps, list(range(N_CORES)), trace=trace)
    return _gather(res.results), res


def kernel(**inputs) -> np.ndarray:
    out, _ = run(trace=False, **inputs)
    return out
